# revision 26
# baseline (speedup 1.0000x reference)
"""Trainium2 Bass kernel for nn_Cluster_Level_GCN (gnn_message_passing).

Collective-free (see kernel2): each core computes its 8 samples' cf rows
directly via host-indexed weighted one-hot matmuls over the ~2k feature rows
per 128-pair chunk (bf16 stream).

v3 fuses the ENTIRE per-chunk pipeline (row stream -> cf -> xT -> Gram ->
topk/adjacency -> agg -> conv -> h1 -> logits -> softmax) into one loop over
the 4 pair-chunks so all compute hides under the DMA row stream.  Every
stage is pair-chunk-local.  PSUM: 4-bank cf accumulator + 4 rotating 1-bank
tiles.
"""

import sys

sys.path.insert(0, "/opt/trn_rl_repo")

import numpy as np

from concourse import bass, mybir, tile
from concourse.bass_utils import run_bass_kernel_spmd
from concourse.masks import make_identity

B, K, N, D = 64, 64, 32768, 2048
NC, NHID, TOPK = 2048, 512, 5
NCORES = 8
SAMP = B // NCORES          # samples per core (8)
R = SAMP * K                # pair rows per core (512)
NCH = D // 128              # d-chunks of 128 (16)
F32 = mybir.dt.float32
BF16 = mybir.dt.bfloat16
AL = mybir.AluOpType

_legal_n = [0]


def _legalize_multiwait(nc):
    """This container's walrus rejects instructions with >1 sync waits
    ("Too many sync wait commands").  Hoist extra waits onto standalone
    single-wait InstEventSemaphore instructions placed just before."""
    for f in nc.m.functions:
        for bb in f.blocks:
            insts = bb.instructions
            if not any(
                i.sync_info is not None and len(i.sync_info.on_wait) > 1
                for i in insts
            ):
                continue
            new = []
            for ins in insts:
                si = ins.sync_info
                if si is not None and len(si.on_wait) > 1:
                    for w in si.on_wait[:-1]:
                        _legal_n[0] += 1
                        new.append(
                            mybir.InstEventSemaphore(
                                name=f"I-lgl-{_legal_n[0]}",
                                ins=[],
                                outs=[],
                                engine=ins.engine,
                                sync_info=mybir.SyncInfo(
                                    on_wait=[w], on_update=[]
                                ),
                            )
                        )
                    ins.sync_info = mybir.SyncInfo(
                        on_wait=[si.on_wait[-1]], on_update=si.on_update
                    )
                new.append(ins)
            bb.instructions = new
    return nc


def build_kernel(nsub):
    """nsub = 128-row subchunks per 128-pair chunk (4 chunks per core)."""
    nc = bass.Bass(trn_type="TRN2", target_bir_lowering=False, debug=False,
                   num_devices=NCORES)

    featrows = nc.dram_tensor("featrows", [4 * nsub * 128, D], BF16,
                              kind="ExternalInput")
    onehotT = nc.dram_tensor("onehotT", [4, 128, nsub * 128], BF16,
                             kind="ExternalInput")
    keep_o = nc.dram_tensor("keep_o", [K, R], F32, kind="ExternalInput")
    cwdev = nc.dram_tensor("cwdev", [8, 128, NCH * 128], BF16,
                           kind="ExternalInput")
    w1 = nc.dram_tensor("w1", [NHID, NHID], BF16, kind="ExternalInput")
    w2p = nc.dram_tensor("w2p", [128, 8], BF16, kind="ExternalInput")
    bias_d = nc.dram_tensor("bias_d", [128, 14], F32, kind="ExternalInput")
    out_d = nc.dram_tensor("out", [1, R * 2], F32, kind="ExternalOutput")

    with tile.TileContext(nc) as tc:
        with (
            tc.tile_pool(name="consts", bufs=1) as cpool,
            tc.tile_pool(name="cf", bufs=2) as cfpool,
            tc.tile_pool(name="xT", bufs=1) as xpool,
            tc.tile_pool(name="agg", bufs=1) as apool,
            tc.tile_pool(name="sB", bufs=2) as sB,
            tc.tile_pool(name="sBig", bufs=2) as sBig,
            tc.tile_pool(name="hh", bufs=1) as hpool,
            tc.tile_pool(name="feat", bufs=16) as fpool,
            tc.tile_pool(name="oh", bufs=1) as opool,
            tc.tile_pool(name="cw", bufs=1) as cwpool,
            tc.tile_pool(name="psA", bufs=1, space="PSUM") as psA,
            tc.tile_pool(name="psB", bufs=2, space="PSUM") as psB,
        ):
            ident = cpool.tile([128, 128], F32)
            make_identity(nc, ident[:])
            ones1 = cpool.tile([1, 128], F32)
            nc.vector.memset(ones1[:], 1.0)
            onec = cpool.tile([128, 1], F32)
            nc.vector.memset(onec[:], 1.0)
            negc = cpool.tile([64, 1], F32)
            nc.vector.memset(negc[:], -1.0)
            zero128 = cpool.tile([128, 128], F32)
            nc.vector.memset(zero128[:], 0.0)
            identb = cpool.tile([128, 128], BF16)
            nc.vector.tensor_copy(out=identb[:], in_=ident[:])

            xT = [xpool.tile([128, R], BF16, tag=f"xT{c}", name=f"xT{c}")
                  for c in range(NCH)]
            aggT = [apool.tile([128, R], BF16, tag=f"ag{c}", name=f"ag{c}")
                    for c in range(NCH)]
            hT = [hpool.tile([128, R], BF16, tag=f"hT{o}", name=f"hT{o}")
                  for o in range(4)]
            h1T = [hpool.tile([128, R], BF16, tag=f"h1T{o}", name=f"h1T{o}")
                   for o in range(4)]

            # one-hot weights + GCN weights (fire at t=0, ahead of the rows)
            oh_t = [opool.tile([128, nsub * 128], BF16, tag=f"oh{p}",
                               name=f"oh{p}") for p in range(4)]
            nc.sync.dma_start(out=oh_t[0][:], in_=onehotT[0, :, :])
            keep_t = sBig.tile([K, R], F32, tag="keep", bufs=1)
            nc.scalar.dma_start(out=keep_t[:], in_=keep_o[:])
            cw_t = [cwpool.tile([128, NCH * 128], BF16, tag=f"cw{oh2}",
                                name=f"cw{oh2}") for oh2 in range(8)]
            for oh2 in range(8):
                nc.scalar.dma_start(out=cw_t[oh2][:], in_=cwdev[oh2, :, :])
            w1_t = [sBig.tile([128, NHID], BF16, tag=f"w1_{c}", bufs=1,
                              name=f"w1t{c}") for c in range(4)]
            for c in range(4):
                nc.scalar.dma_start(out=w1_t[c][:],
                                    in_=w1[c * 128:(c + 1) * 128, :])
            w2_t = sB.tile([128, 8], BF16, tag="w2", bufs=1)
            nc.scalar.dma_start(out=w2_t[:], in_=w2p[:])
            bias_t = sB.tile([128, 14], F32, tag="bias", bufs=1)
            nc.scalar.dma_start(out=bias_t[:], in_=bias_d[:])

            # persistent small state
            A5all = sBig.tile([64, R], F32, tag="A5all", bufs=1)
            adjMD = sBig.tile([128, R], BF16, tag="adjMD", bufs=1)
            nc.vector.memset(adjMD[:], 0.0)
            negs = sB.tile([1, R], F32, tag="negs", bufs=1)
            invnrow = sB.tile([1, R], F32, tag="invnrow", bufs=1)
            ball_sb = sBig.tile([128, R], F32, tag="ballsb", bufs=1)
            outt = sB.tile([1, 2 * R], F32, tag="outt", bufs=1)

            for p in range(4):
                pl = slice(p * 128, (p + 1) * 128)
                # ---- phase A: stream rows, accumulate cf chunk ----
                if p + 1 < 4:  # prefetch next chunk's one-hot
                    nc.sync.dma_start(out=oh_t[p + 1][:],
                                      in_=onehotT[p + 1, :, :])
                ps = psA.tile([128, D], F32, tag="psA", name=f"psA{p}")
                for ss in range(nsub):
                    ft = fpool.tile([128, D], BF16)
                    nc.sync.dma_start(
                        out=ft[:],
                        in_=featrows[(p * nsub + ss) * 128:
                                     (p * nsub + ss + 1) * 128, :])
                    for seg in range(4):
                        nc.tensor.matmul(
                            ps[:, seg * 512:(seg + 1) * 512],
                            lhsT=oh_t[p][:, ss * 128:(ss + 1) * 128],
                            rhs=ft[:, seg * 512:(seg + 1) * 512],
                            start=(ss == 0), stop=(ss == nsub - 1))
                cf_c = cfpool.tile([128, D], BF16, tag="cfc", name=f"cf{p}")
                for seg in range(4):
                    nc.vector.tensor_copy(
                        out=cf_c[:, seg * 512:(seg + 1) * 512],
                        in_=ps[:, seg * 512:(seg + 1) * 512])
                    for c in range(4 * seg, 4 * seg + 4):
                        if p < 3:
                            # keep the DMA device free mid-stream: PE
                            # transpose + gpsimd copy-out instead
                            tp_ps = psB.tile([128, 128], BF16, tag="ps1")
                            nc.tensor.transpose(
                                out=tp_ps[:],
                                in_=cf_c[:, c * 128:(c + 1) * 128],
                                identity=identb[:])
                            nc.scalar.activation(
                                out=xT[c][:, pl], in_=tp_ps[:],
                                func=mybir.ActivationFunctionType.Copy)
                        else:
                            nc.scalar.dma_start_transpose(
                                out=xT[c][:, pl],
                                in_=cf_c[:, c * 128:(c + 1) * 128])

                # ---- Gram (both samples of the chunk at once) ----
                if p == 3:
                    # use a free region of the cf accumulator banks so the
                    # tail never waits on psB rotation
                    A2 = ps[:, 128:256]
                else:
                    A2 = psB.tile([128, 128], F32, tag="ps")
                for c in range(NCH):
                    nc.tensor.matmul(A2[:], lhsT=xT[c][:, pl],
                                     rhs=xT[c][:, pl],
                                     start=(c == 0), stop=(c == NCH - 1))
                for h in range(2):
                    s = 2 * p + h
                    blk = A2[h * 64:(h + 1) * 64, h * 64:(h + 1) * 64]
                    nc.vector.tensor_scalar(
                        out=A5all[:, s * K:(s + 1) * K], in0=blk,
                        scalar1=0.2, scalar2=None, op0=AL.mult)

                # norms: Gram diag as a ROW (ones-column contraction of the
                # identity-masked Gram), then invnrow = 1/sqrt(diag) in one
                # activation; column form via a single PE transpose
                dg = sB.tile([128, 128], F32, tag="dg")
                nc.vector.tensor_tensor(out=dg[:], in0=A2[:], in1=ident[:],
                                        op=AL.mult)
                nsqr_ps = psB.tile([1, 128], F32, tag="ps1")
                nc.tensor.matmul(nsqr_ps[:], lhsT=onec[:], rhs=dg[:],
                                 start=True, stop=True)
                nrow = sB.tile([1, 128], F32, tag="nrow")
                nc.scalar.activation(
                    out=nrow[:], in_=nsqr_ps[:],
                    func=mybir.ActivationFunctionType.Sqrt)
                if p == 3:
                    # preload the Sigmoid table right after the LAST Sqrt so
                    # the tail's sigmoids find it warm
                    dummy = sB.tile([1, 1], F32, tag="dummy", bufs=1)
                    nc.scalar.activation(
                        out=dummy[:], in_=nrow[0:1, 0:1],
                        func=mybir.ActivationFunctionType.Sigmoid)
                nc.vector.reciprocal(out=invnrow[:, pl], in_=nrow[:])
                invc_ps = psB.tile([128, 1], F32, tag="ps1")
                nc.tensor.transpose(out=invc_ps[:], in_=invnrow[:, pl],
                                    identity=ident[:1, :1])
                invncol = sB.tile([128, 1], F32, tag="invncol")
                nc.vector.tensor_copy(out=invncol[:], in_=invc_ps[:])
                ball_ps = psB.tile([128, 128], F32, tag="ps")
                nc.tensor.matmul(ball_ps[:], lhsT=ones1[:],
                                 rhs=invnrow[:, pl], start=True, stop=True)
                nc.vector.tensor_copy(out=ball_sb[:, pl], in_=ball_ps[:])

                # ---- topk mask + adjacency (per sample) ----
                for h in range(2):
                    s = 2 * p + h
                    sl = slice(s * K, (s + 1) * K)
                    bs = h * 64
                    t8 = sB.tile([64, 8], F32, tag="t8")
                    nc.vector.max(out=t8[:], in_=A5all[:, sl])
                    m0 = sB.tile([64, 64], F32, tag="m0")
                    nc.vector.tensor_scalar(
                        out=m0[:], in0=A5all[:, sl],
                        scalar1=t8[:, TOPK - 1:TOPK],
                        scalar2=None, op0=AL.is_ge)
                    m0T_ps = psB.tile([64, 64], F32, tag="ps1")
                    nc.tensor.transpose(out=m0T_ps[:], in_=m0[:],
                                        identity=ident[:64, :64])
                    msym = sB.tile([64, 64], F32, tag="msym")
                    nc.vector.tensor_tensor(out=msym[:], in0=m0[:],
                                            in1=m0T_ps[:], op=AL.mult)
                    nc.vector.tensor_tensor(out=msym[:], in0=msym[:],
                                            in1=keep_t[:, sl], op=AL.mult)
                    msk5 = sB.tile([64, 64], F32, tag="msk5")
                    nc.vector.tensor_tensor(out=msk5[:], in0=A5all[:, sl],
                                            in1=msym[:], op=AL.mult)
                    nc.vector.tensor_scalar(
                        out=adjMD[bs:bs + 64, sl], in0=msk5[:],
                        scalar1=invncol[bs:bs + 64, 0:1],
                        scalar2=None, op0=AL.mult)
                    negs_ps = psB.tile([1, 64], F32, tag="ps1")
                    nc.tensor.matmul(negs_ps[:], lhsT=negc[:], rhs=msk5[:],
                                     start=True, stop=True)
                    r0 = sB.tile([1, 64], F32, tag="r0")
                    nc.vector.tensor_tensor(out=r0[:], in0=msk5[0:1, :],
                                            in1=negs_ps[:], op=AL.add)
                    nc.vector.tensor_scalar_mul(
                        adjMD[bs:bs + 1, sl], r0[:],
                        invncol[bs:bs + 1, 0:1])

                # chunk 3: x-half conv early into the freed cf accumulator
                # banks -- PE stays busy through the topk/adjacency chain
                phx3 = []
                if p == 3:
                    for o in range(4):
                        reg = ps[:, o * 512:o * 512 + 128]
                        for ci in range(NCH):
                            nc.tensor.matmul(
                                reg,
                                lhsT=cw_t[2 * o][:, ci * 128:(ci + 1) * 128],
                                rhs=xT[ci][:, pl],
                                start=(ci == 0), stop=(ci == NCH - 1))
                        phx3.append(reg)

                # ---- agg ----
                for c in range(NCH):
                    ag_ps = psB.tile([128, 128], F32, tag="ps")
                    nc.tensor.matmul(
                        ag_ps[:],
                        lhsT=cf_c[:, c * 128:(c + 1) * 128],
                        rhs=adjMD[:, pl], start=True, stop=True)
                    nc.vector.tensor_copy(out=aggT[c][:, pl], in_=ag_ps[:])

                # ---- conv ----
                for o in range(4):
                    if p == 3:
                        phx = phx3[o]
                        pha = psB.tile([128, 128], F32, tag="ps")
                    else:
                        ph2 = psB.tile([128, 256], F32, tag="ps")
                        phx, pha = ph2[:, 0:128], ph2[:, 128:256]
                        for ci in range(NCH):
                            nc.tensor.matmul(
                                phx, lhsT=cw_t[2 * o][:, ci * 128:
                                                      (ci + 1) * 128],
                                rhs=xT[ci][:, pl],
                                start=(ci == 0), stop=(ci == NCH - 1))
                    for ci in range(NCH):
                        nc.tensor.matmul(
                            pha, lhsT=cw_t[2 * o + 1][:, ci * 128:
                                                      (ci + 1) * 128],
                            rhs=aggT[ci][:, pl],
                            start=(ci == 0), stop=(ci == NCH - 1))
                    # x-half rescaled out of PSUM (walrus: DVE reads at
                    # most one PSUM operand per instruction)
                    xh_sb = sB.tile([128, 128], F32, tag="xh")
                    nc.vector.tensor_tensor(out=xh_sb[:], in0=phx,
                                            in1=ball_sb[:, pl], op=AL.mult)
                    bf = sB.tile([128, 2], F32, tag="bf")
                    nc.vector.tensor_tensor(
                        out=bf[:, :, None],
                        in0=bias_t[:, o:o + 1].to_broadcast(
                            [128, 2])[:, :, None],
                        in1=xh_sb[:].rearrange("q (s k) -> q s k",
                                               k=K)[:, :, 0:1],
                        op=AL.subtract)
                    for h in range(2):
                        s = 2 * p + h
                        hsl = slice(h * 64, (h + 1) * 64)
                        tmp = sB.tile([128, 64], F32, tag="tmph")
                        nc.vector.scalar_tensor_tensor(
                            out=tmp[:], in0=pha[:, hsl],
                            scalar=bf[:, h:h + 1],
                            in1=xh_sb[:, hsl], op0=AL.add, op1=AL.add)
                        nc.vector.tensor_scalar(
                            out=hT[o][:, s * K:(s + 1) * K],
                            in0=tmp[:], scalar1=0.0,
                            scalar2=None, op0=AL.max)

                # ---- h1 (w1 @ h, PReLU) ----
                for o in range(4):
                    ph1 = psB.tile([128, 128], F32, tag="ps")
                    for c in range(4):
                        nc.tensor.matmul(
                            ph1[:],
                            lhsT=w1_t[c][:, o * 128:(o + 1) * 128],
                            rhs=hT[c][:, pl],
                            start=(c == 0), stop=(c == 3))
                    pos = sB.tile([128, 128], F32, tag="pos")
                    nc.vector.tensor_scalar(out=pos[:], in0=ph1[:],
                                            scalar1=bias_t[:, 4 + o:5 + o],
                                            scalar2=0.0, op0=AL.add,
                                            op1=AL.max)
                    pre = sB.tile([128, 128], F32, tag="pre")
                    nc.vector.scalar_tensor_tensor(
                        out=pre[:], in0=ph1[:],
                        scalar=bias_t[:, 4 + o:5 + o],
                        in1=zero128[:], op0=AL.add, op1=AL.min)
                    nc.vector.scalar_tensor_tensor(
                        out=h1T[o][:, pl], in0=pre[:],
                        scalar=bias_t[:, 8 + o:9 + o],
                        in1=pos[:], op0=AL.mult, op1=AL.add)

            # ---- tail: logits + softmax over all chunks ----
            pl0 = psB.tile([1, R], F32, tag="ps1")
            pl1 = psB.tile([1, R], F32, tag="ps1")
            for c in range(4):
                nc.tensor.matmul(pl0[:], lhsT=w2_t[:, 2 * c:2 * c + 1],
                                 rhs=h1T[c][:],
                                 start=(c == 0), stop=(c == 3))
                nc.tensor.matmul(pl1[:],
                                 lhsT=w2_t[:, 2 * c + 1:2 * c + 2],
                                 rhs=h1T[c][:],
                                 start=(c == 0), stop=(c == 3))
            lg0 = sB.tile([1, R], F32, tag="lg0", bufs=1)
            lg1 = sB.tile([1, R], F32, tag="lg1", bufs=1)
            nc.vector.tensor_scalar(out=lg0[:], in0=pl0[:],
                                    scalar1=bias_t[0:1, 12:13], scalar2=None,
                                    op0=AL.add)
            nc.vector.tensor_scalar(out=lg1[:], in0=pl1[:],
                                    scalar1=bias_t[0:1, 13:14], scalar2=None,
                                    op0=AL.add)
            dl = sB.tile([1, R], F32, tag="dl", bufs=1)
            nc.vector.tensor_tensor(out=dl[:], in0=lg0[:], in1=lg1[:],
                                    op=AL.subtract)
            o3 = outt[:].rearrange("q (k c) -> q k c", c=2)
            # softmax([l0,l1]) = [sigmoid(dl), sigmoid(-dl)], written strided
            nc.scalar.activation(out=o3[:, :, 0:1], in_=dl[:, :, None],
                                 func=mybir.ActivationFunctionType.Sigmoid)
            nc.scalar.activation(out=o3[:, :, 1:2], in_=dl[:, :, None],
                                 func=mybir.ActivationFunctionType.Sigmoid,
                                 scale=-1.0)
            nc.sync.dma_start(out=out_d[:], in_=outt[:])

    _legalize_multiwait(nc)
    return nc


# ---------------------------------------------------------------------------
# host side (index-only work + dtype casts) — same as kernel2
# ---------------------------------------------------------------------------

def _preprocess(indexes, features, labels, ori_knn_neighbor,
                conv_w, conv_b, w1, b1, prelu_a, w2, b2):
    bf = mybir.dt.np(BF16)
    indexes = np.asarray(indexes).astype(np.int64)
    labels = np.asarray(labels).astype(np.int64)
    nbr = np.asarray(ori_knn_neighbor).astype(np.int64)
    features = np.asarray(features, dtype=np.float32)
    fbf = features.astype(bf)

    counts = np.bincount(labels, minlength=NC)
    wcnt = (1.0 / np.maximum(counts, 1)).astype(np.float32)

    order = np.argsort(labels, kind="stable")
    slab = labels[order]
    starts = np.searchsorted(slab, np.arange(NC + 1))

    clu_lab = labels[nbr]                        # [B, K]
    keep = np.ones((B, K), dtype=np.float32)
    for b in range(B):
        seen = set()
        for k in range(K):
            l = int(clu_lab[b, k])
            if l in seen:
                keep[b, k] = 0.0
            else:
                seen.add(l)

    # balance pair assignment (largest-with-smallest by row count) to
    # minimize the global max rows/chunk (=> smaller nsub); the output is
    # un-permuted in kernel()
    rows_per_sample = np.array([
        counts[np.unique(clu_lab[b])].sum() for b in range(B)])
    perms = []
    for core in range(NCORES):
        ss = sorted(range(core * SAMP, (core + 1) * SAMP),
                    key=lambda x: rows_per_sample[x])
        perm = []
        for i in range(4):
            perm += [ss[i], ss[SAMP - 1 - i]]
        perms.append(perm)          # local slot -> global sample
    chunk_rows = {}
    for core in range(NCORES):
        for p in range(4):
            s0, s1 = perms[core][2 * p], perms[core][2 * p + 1]
            uniq = np.unique(clu_lab[[s0, s1]])
            row_list = np.concatenate(
                [order[starts[u]:starts[u + 1]] for u in uniq])
            chunk_rows[(core, p)] = (uniq, row_list)
    nsub = max((len(rl) + 2 + 127) // 128 for _, rl in chunk_rows.values())

    cwdev = np.ascontiguousarray(
        np.asarray(conv_w, dtype=np.float32)
        .reshape(2, 16, 128, 4, 128)
        .transpose(3, 0, 2, 1, 4)
        .reshape(8, 128, 16 * 128)).astype(bf)
    w1_b = np.ascontiguousarray(np.asarray(w1, dtype=np.float32)).astype(bf)
    w2_b = np.ascontiguousarray(
        np.asarray(w2, dtype=np.float32).reshape(4, 128, 2)
        .transpose(1, 0, 2).reshape(128, 8)).astype(bf)
    bias_h = np.zeros((128, 14), np.float32)
    bias_h[:, 0:4] = np.asarray(conv_b, dtype=np.float32).reshape(4, 128).T
    bias_h[:, 4:8] = np.asarray(b1, dtype=np.float32).reshape(4, 128).T
    bias_h[:, 8:12] = np.asarray(prelu_a, dtype=np.float32).reshape(4, 128).T
    bias_h[0, 12:14] = np.asarray(b2, dtype=np.float32)

    per_core = []
    for core in range(NCORES):
        fr = np.zeros((4 * nsub * 128, D), dtype=bf)
        oh = np.zeros((4, 128, nsub * 128), dtype=np.float32)
        for p in range(4):
            s0, s1 = perms[core][2 * p], perms[core][2 * p + 1]
            uniq, row_list = chunk_rows[(core, p)]
            rows = np.concatenate([row_list, indexes[[s0, s1]]])
            nr = len(rows)
            fr[p * nsub * 128: p * nsub * 128 + nr] = fbf[rows]
            cnts = starts[uniq + 1] - starts[uniq]
            cstart = np.concatenate([[0], np.cumsum(cnts)])
            for j in range(128):
                s = s0 if j < 64 else s1
                k = j % 64
                if k == 0:
                    qpos = len(row_list) + (0 if j < 64 else 1)
                    oh[p, qpos % 128, (qpos // 128) * 128 + j] = 1.0
                else:
                    lab = clu_lab[s, k]
                    t = np.searchsorted(uniq, lab)
                    rr = np.arange(cstart[t], cstart[t + 1])
                    oh[p, rr % 128, (rr // 128) * 128 + j] = wcnt[lab]

        ko = np.empty((K, R), dtype=np.float32)
        for si in range(SAMP):
            b = perms[core][si]
            ko[:, si * K:(si + 1) * K] = np.outer(keep[b], keep[b])

        per_core.append(dict(
            featrows=fr,
            onehotT=oh.astype(bf),
            keep_o=ko,
            cwdev=cwdev,
            w1=w1_b,
            w2p=w2_b,
            bias_d=bias_h,
        ))
    return per_core, nsub, perms


_cache = {}


def kernel(indexes, features, labels, ori_knn_neighbor,
           conv_w, conv_b, w1, b1, prelu_a, w2, b2):
    in_maps, nsub, perms = _preprocess(
        indexes, features, labels, ori_knn_neighbor,
        conv_w, conv_b, w1, b1, prelu_a, w2, b2)
    if nsub not in _cache:
        _cache[nsub] = build_kernel(nsub)
    nc = _cache[nsub]
    res = run_bass_kernel_spmd(nc, in_maps, core_ids=list(range(NCORES)))
    out = np.empty((B, K, 2), np.float32)
    for c in range(NCORES):
        loc = res.results[c]["out"].reshape(SAMP, K, 2)
        for si in range(SAMP):
            out[perms[c][si]] = loc[si]
    return out
